# revision 1
# baseline (speedup 1.0000x reference)
"""Trainium2 Bass kernel for nn_CHI_Block (dual cross-attention transformer block).

Sharding: data-parallel over batch — B=8 batch elements -> 8 NeuronCores, no
collectives. Per core: full block on one [1024, 1024] slice.

Layout strategy (per core):
  - LayerNorms computed token-major (reductions on free axis), gains folded
    into the following weight matrices host-side, biases folded into
    per-feature bias vectors applied on-chip.
  - Activations kept feature-major (x^T) for matmul chaining; produced by
    PE transposes (plain matmul against identity).
  - q,k feature-major (weights stationary); v token-major (x^T stationary).
  - Attention uses transposed scores S^T[m,n] = k^T stationary @ q^T moving,
    so softmax'd scores feed the A@V matmul directly with no transposes.
    Softmax has no max-subtraction (logits are O(1) by construction); the
    normalizer is computed by a ones-matmul over exp(S^T) on the PE and
    folded in at the attention-output eviction.
  - 2 heads (d=64) packed per PE pass: row-packing (K=64+64) for scores,
    column-packing (M=64+64) for A@V.
  - All matmuls bf16 inputs with fp32 PSUM accumulation; residual trunk fp32
    in DRAM.
"""
import sys

for _p in ("/opt/trn_rl_repo",):
    if _p not in sys.path:
        sys.path.insert(0, _p)

import numpy as np
import ml_dtypes

import concourse.bass as bass
import concourse.tile as tile
from concourse import bacc, mybir
from concourse.bass_utils import run_bass_kernel_spmd
from concourse.masks import make_identity

F32 = mybir.dt.float32
BF16 = mybir.dt.bfloat16
LN_EPS = 1e-5
N_CORES = 8


def build_nc(B_N=1024, C=1024, H=16, HID=4096, n_iters=1, act="Gelu",
             v_aug=False, no_attn=False, no_mlp=False, no_ln=False,
             no_proj=False, no_wp=False):
    """Build the per-core Bass program. All cores run the same program (SPMD)."""
    d = C // H
    assert d == 64 and H % 2 == 0
    TN = B_N // 128          # token tiles
    TC = C // 128            # feature tiles
    TH = HID // 128          # hidden tiles
    CHUNK = min(512, B_N)    # moving free-dim chunk (tokens)
    NCH = B_N // CHUNK
    WCH = min(512, C)        # moving free-dim chunk (features)
    NWC = C // WCH
    HCH = min(512, HID)
    NHC = HID // HCH
    BNF = min(512, C)        # bn_stats max free
    PAIRS = H // 2

    nc = bacc.Bacc("TRN2", target_bir_lowering=False, debug=False,
                   num_devices=N_CORES)

    def din(name, shape, dt=BF16):
        return nc.dram_tensor(name, shape, dt, kind="ExternalInput").ap()

    x1 = din("x1", [B_N, C], F32)
    x2 = din("x2", [B_N, C], F32)
    x3 = din("x3", [B_N, C], F32)
    wq = [din("wq1", [C, C]), din("wq2", [C, C])]
    wk = [din("wk1", [C, C]), din("wk2", [C, C])]
    wv = [din("wv1", [C, C]), din("wv2", [C, C])]
    wp = [din("wp1", [C, C]), din("wp2", [C, C])]
    qb = [din("qb1", [C], F32), din("qb2", [C], F32)]
    kb = [din("kb1", [C], F32), din("kb2", [C], F32)]
    battn = din("battn", [C])      # bf16 row, added once (branch 1)
    wfc1 = din("wfc1", [C, HID])
    wfc2 = din("wfc2", [HID, C])
    hb = din("hb", [HID], F32)
    fbias = din("fbias", [C])      # bf16 row
    out = nc.dram_tensor("out", [B_N, C], F32, kind="ExternalOutput").ap()
    x_mid = nc.dram_tensor("x_mid", [B_N, C], F32).ap()   # residual trunk

    ACT = mybir.ActivationFunctionType
    ALU = mybir.AluOpType

    with tile.TileContext(nc) as tc:
        from contextlib import nullcontext
        loop = tc.For_i(0, n_iters, 1) if n_iters > 1 else nullcontext()
        with loop:
            _body(nc, tc, locals())
    nc.compile()
    return nc


def _body(nc, tc, g):
    """Kernel body. `g` carries everything from build_nc's scope."""
    from contextlib import ExitStack

    (B_N, C, H, HID, d, TN, TC, TH, CHUNK, NCH, WCH, NWC, HCH, NHC, BNF,
     PAIRS) = (g[k] for k in
               ("B_N", "C", "H", "HID", "d", "TN", "TC", "TH", "CHUNK", "NCH",
                "WCH", "NWC", "HCH", "NHC", "BNF", "PAIRS"))
    v_aug = g["v_aug"]
    no_attn, no_mlp, no_ln, no_proj, no_wp = (
        g[k] for k in ("no_attn", "no_mlp", "no_ln", "no_proj", "no_wp"))
    HPC = WCH // 64  # heads per v-projection chunk
    x1, x2, x3, wq, wk, wv, wp, qb, kb, battn = (g[k] for k in
        ("x1", "x2", "x3", "wq", "wk", "wv", "wp", "qb", "kb", "battn"))
    wfc1, wfc2, hb, fbias, out, x_mid = (g[k] for k in
        ("wfc1", "wfc2", "hb", "fbias", "out", "x_mid"))
    ACT = mybir.ActivationFunctionType
    ALU = mybir.AluOpType

    with ExitStack() as ctx:
        constp = ctx.enter_context(tc.tile_pool(name="const", bufs=1))
        glob = ctx.enter_context(tc.tile_pool(name="glob", bufs=1))
        work = ctx.enter_context(tc.tile_pool(name="work", bufs=2 if v_aug else 3))
        statsp = ctx.enter_context(tc.tile_pool(name="stats", bufs=8))

        # ---- constants ----
        ident = constp.tile([128, 128], BF16)
        make_identity(nc, ident)
        ones = constp.tile([128, 128], BF16)
        nc.vector.memset(ones, 1.0)
        eps = constp.tile([128, 1], F32)
        nc.vector.memset(eps, LN_EPS)
        qb_sb = [constp.tile([128, TC], F32, tag=f"qb{i}", name=f"qb_sb{i}")
                 for i in range(2)]
        kb_sb = [constp.tile([128, TC], F32, tag=f"kb{i}", name=f"kb_sb{i}")
                 for i in range(2)]
        for i in range(2):
            nc.sync.dma_start(out=qb_sb[i], in_=qb[i].rearrange("(j p) -> p j", p=128))
            nc.sync.dma_start(out=kb_sb[i], in_=kb[i].rearrange("(j p) -> p j", p=128))
        hb_sb = constp.tile([128, TH], F32)
        nc.sync.dma_start(out=hb_sb, in_=hb.rearrange("(j p) -> p j", p=128))
        battn_sb = constp.tile([1, C], BF16)
        nc.sync.dma_start(out=battn_sb, in_=battn[None, :])
        fbias_sb = constp.tile([1, C], BF16)
        nc.sync.dma_start(out=fbias_sb, in_=fbias[None, :])
        zrow = constp.tile([1, C], BF16)
        nc.vector.memset(zrow, 0.0)

        # persistent feature-major LN'd inputs
        xn1T = glob.tile([128, TC, B_N], BF16, tag="xn1T")
        xnkvT = [glob.tile([128, TC, B_N], BF16, tag="xnkvT", name="xn2T")]
        if no_ln:
            nc.vector.memset(xn1T, 0.125)
            nc.vector.memset(xnkvT[0], 0.125)

        def layer_norm_T(src_dram, dstT, ps_t):
            """token-major LN of src -> feature-major bf16 dstT [128, TC, B_N]."""
            if no_ln:
                return
            for t in range(TN):
                xt = work.tile([128, C], F32, tag="ln_x")
                nc.sync.dma_start(out=xt, in_=src_dram[t * 128:(t + 1) * 128, :])
                st = statsp.tile([128, C // BNF, 6], F32, tag="bn6")
                for s in range(C // BNF):
                    nc.vector.bn_stats(out=st[:, s, :],
                                       in_=xt[:, s * BNF:(s + 1) * BNF])
                mv = statsp.tile([128, 2], F32, tag="mv")
                nc.vector.bn_aggr(out=mv, in_=st)
                sd = statsp.tile([128, 1], F32, tag="sd")
                nc.scalar.activation(out=sd, in_=mv[:, 1:2], func=ACT.Sqrt,
                                     bias=eps, scale=1.0)
                r = statsp.tile([128, 1], F32, tag="r")
                nc.vector.reciprocal(out=r, in_=sd)
                nm = statsp.tile([128, 1], F32, tag="nm")
                nc.vector.tensor_scalar(out=nm, in0=mv[:, 0:1], scalar1=r,
                                        scalar2=-1.0, op0=ALU.mult, op1=ALU.mult)
                xn = work.tile([128, C], BF16, tag="ln_xn")
                nc.vector.tensor_scalar(out=xn, in0=xt, scalar1=r, scalar2=nm,
                                        op0=ALU.mult, op1=ALU.add)
                for j in range(TC):
                    pt = ps_t.tile([128, 128], F32, tag="ps_t")
                    nc.tensor.matmul(pt, xn[:, j * 128:(j + 1) * 128], ident,
                                     start=True, stop=True)
                    nc.vector.tensor_copy(
                        dstT[:, j, t * 128:(t + 1) * 128], pt)

        with tc.tile_pool(name="ps_ln", bufs=4, space="PSUM") as ps_t:
            layer_norm_T(x1, xn1T, ps_t)
            layer_norm_T(x2, xnkvT[0], ps_t)

        # ---- branches ----
        for br in range(2):
            with ExitStack() as bctx:
                attnp = bctx.enter_context(tc.tile_pool(name=f"attn{br}", bufs=1))
                wpool = bctx.enter_context(
                    tc.tile_pool(name=f"w{br}", bufs=1 if v_aug else 2))
                xkvT = xnkvT[br]

                qT = attnp.tile([128, TC, B_N], BF16, tag="qT")
                kT = attnp.tile([128, TC, B_N], BF16, tag="kT")
                if v_aug:
                    V = attnp.tile([128, TN, H, 128], BF16, tag="V")
                    nc.vector.memset(V[:, :, :, 64:128], 1.0)
                else:
                    V = attnp.tile([128, TN, C], BF16, tag="V")
                OT = attnp.tile([128, TC, B_N], BF16, tag="OT")

                # --- q/k projections (feature-major: W stationary) ---
                with tc.tile_pool(name=f"ps_p{br}", bufs=6, space="PSUM") as psp:
                    for dstT, srcT, w_dram, b_sb in () if no_proj else (
                            (qT, xn1T, wq[br], qb_sb[br]),
                            (kT, xkvT, wk[br], kb_sb[br])):
                        w_sb = wpool.tile([128, TC, C], BF16, tag="W")
                        wg = w_dram.rearrange("(k p) c -> p k c", p=128)
                        for k in range(TC):
                            nc.sync.dma_start(out=w_sb[:, k, :], in_=wg[:, k, :])
                        for j in range(TC):
                            for ch in range(NCH):
                                ps = psp.tile([128, CHUNK], F32, tag="ps")
                                for k in range(TC):
                                    nc.tensor.matmul(
                                        ps, w_sb[:, k, j * 128:(j + 1) * 128],
                                        srcT[:, k, ch * CHUNK:(ch + 1) * CHUNK],
                                        start=(k == 0), stop=(k == TC - 1))
                                nc.vector.tensor_scalar(
                                    out=dstT[:, j, ch * CHUNK:(ch + 1) * CHUNK],
                                    in0=ps, scalar1=b_sb[:, j:j + 1], scalar2=None,
                                    op0=ALU.add)
                    # --- v projection (token-major: x^T stationary) ---
                    w_sb = wpool.tile([128, TC, C], BF16, tag="W")
                    wg = wv[br].rearrange("(k p) c -> p k c", p=128)
                    for k in range(TC):
                        nc.sync.dma_start(out=w_sb[:, k, :], in_=wg[:, k, :])
                    for mt in range(0 if no_proj else TN):
                        for ch in range(NWC):
                            ps = psp.tile([128, WCH], F32, tag="ps")
                            for k in range(TC):
                                nc.tensor.matmul(
                                    ps, xkvT[:, k, mt * 128:(mt + 1) * 128],
                                    w_sb[:, k, ch * WCH:(ch + 1) * WCH],
                                    start=(k == 0), stop=(k == TC - 1))
                            if v_aug:
                                nc.vector.tensor_copy(
                                    V[:, mt, ch * HPC:(ch + 1) * HPC, 0:64], ps)
                            else:
                                nc.vector.tensor_copy(
                                    V[:, mt, ch * WCH:(ch + 1) * WCH], ps)
                if br == 0:
                    xnkvT.append(glob.tile([128, TC, B_N], BF16, tag="xnkvT",
                                           name="xn3T"))
                    if no_ln:
                        nc.vector.memset(xnkvT[1], 0.125)
                    with tc.tile_pool(name="ps_ln2", bufs=4, space="PSUM") as pst2:
                        layer_norm_T(x3, xnkvT[1], pst2)
                if no_proj:
                    nc.vector.memset(qT, 0.125)
                    nc.vector.memset(kT, 0.125)
                    nc.vector.memset(V, 0.125)
                if no_attn:
                    nc.vector.memset(OT, 0.125)

                # --- attention, head pairs ---
                # m-halving + double-buffered E measured slower on HW than the
                # plain full-E single-buffer layout; HTN=TN restores the latter
                # for the sums path (v_aug still uses TN//2 halves).
                HTN = TN // 2 if v_aug else TN
                EH_BUFS = 2 if v_aug else 1

                def eh_tiles():
                    return (attnp.tile([128, HTN, B_N], BF16, tag="EA", name="EAh",
                                       bufs=EH_BUFS),
                            attnp.tile([128, HTN, B_N], BF16, tag="EB", name="EBh",
                                       bufs=EH_BUFS))

                def e_tiles():
                    return (attnp.tile([128, TN, B_N], BF16, tag="EA", name="EA"),
                            attnp.tile([128, TN, B_N], BF16, tag="EB", name="EB"))

                def r_tiles():
                    return (attnp.tile([128, B_N], F32, tag="RA", name="RA"),
                            attnp.tile([128, B_N], F32, tag="RB", name="RB"))

                ps_o_bufs = 4 if v_aug else 2
                with (tc.tile_pool(name=f"ps_s{br}", bufs=2, space="PSUM") as ps_s,
                      tc.tile_pool(name=f"ps_o{br}", bufs=ps_o_bufs,
                                   space="PSUM") as ps_o,
                      ExitStack() as actx):
                    if not v_aug:
                        ps_n = actx.enter_context(
                            tc.tile_pool(name=f"ps_n{br}", bufs=2, space="PSUM"))
                    for p in range(0 if no_attn else PAIRS):
                        hA, hB = 2 * p, 2 * p + 1
                        qp = qT[:, p, :]
                        kp = kT[:, p, :]
                        if v_aug:
                            # AV accumulators for the whole pair (4 = all of
                            # ps_o); rows 64-127 = softmax sums (ones-padded V)
                            psav = {(h, ch): ps_o.tile([128, CHUNK], F32,
                                                       tag="pso",
                                                       name=f"psav{h % 2}{ch}")
                                    for h in (hA, hB) for ch in range(NCH)}
                            for half in range(2):
                                EA, EB = eh_tiles()
                                for mtl in range(HTN):
                                    mt = half * HTN + mtl
                                    psA = ps_s.tile([128, B_N], F32, tag="psS",
                                                    name="psA")
                                    psB = ps_s.tile([128, B_N], F32, tag="psS",
                                                    name="psB")
                                    for ch in range(NCH):
                                        sl = slice(ch * CHUNK, (ch + 1) * CHUNK)
                                        nc.tensor.matmul(
                                            psA[:, sl],
                                            kp[0:64, mt * 128:(mt + 1) * 128],
                                            qp[0:64, sl], start=True, stop=True)
                                        nc.tensor.matmul(
                                            psB[:, sl],
                                            kp[64:128, mt * 128:(mt + 1) * 128],
                                            qp[64:128, sl], start=True, stop=True)
                                    nc.scalar.activation(
                                        out=EA[:, mtl, :], in_=psA, func=ACT.Exp,
                                        scale=float(d) ** -0.5)
                                    nc.scalar.activation(
                                        out=EB[:, mtl, :], in_=psB, func=ACT.Exp,
                                        scale=float(d) ** -0.5)
                                for h, E in ((hA, EA), (hB, EB)):
                                    for ch in range(NCH):
                                        sl = slice(ch * CHUNK, (ch + 1) * CHUNK)
                                        for mtl in range(HTN):
                                            mt = half * HTN + mtl
                                            nc.tensor.matmul(
                                                psav[(h, ch)], V[:, mt, h, :],
                                                E[:, mtl, sl],
                                                start=(mt == 0),
                                                stop=(mt == TN - 1))
                            for h, orows in ((hA, slice(0, 64)),
                                             (hB, slice(64, 128))):
                                for ch in range(NCH):
                                    sl = slice(ch * CHUNK, (ch + 1) * CHUNK)
                                    ps = psav[(h, ch)]
                                    rsb = statsp.tile([64, CHUNK], F32, tag="rsb")
                                    nc.vector.reciprocal(out=rsb, in_=ps[64:128, :])
                                    nc.vector.tensor_tensor(
                                        out=OT[orows, p, sl], in0=ps[0:64, :],
                                        in1=rsb, op=ALU.mult)
                            continue
                        RA, RB = r_tiles()
                        EH = []  # [(EA_half, EB_half)] — halves double-buffer
                        # scores S^T + exp, row-packed head pair
                        for mt in range(TN):
                            if mt % HTN == 0:
                                EH.append(eh_tiles())
                            EA, EB = EH[-1]
                            mtl = mt % HTN
                            psA = ps_s.tile([128, B_N], F32, tag="psS", name="psA")
                            psB = ps_s.tile([128, B_N], F32, tag="psS", name="psB")
                            for ch in range(NCH):
                                sl = slice(ch * CHUNK, (ch + 1) * CHUNK)
                                nc.tensor.matmul(
                                    psA[:, sl], kp[0:64, mt * 128:(mt + 1) * 128],
                                    qp[0:64, sl], start=True, stop=True)
                                nc.tensor.matmul(
                                    psB[:, sl], kp[64:128, mt * 128:(mt + 1) * 128],
                                    qp[64:128, sl], start=True, stop=True)
                            nc.scalar.activation(out=EA[:, mtl, :], in_=psA,
                                                 func=ACT.Exp, scale=float(d) ** -0.5)
                            nc.scalar.activation(out=EB[:, mtl, :], in_=psB,
                                                 func=ACT.Exp, scale=float(d) ** -0.5)

                        def eslice(x, mt, sl):
                            return EH[mt // HTN][x][:, mt % HTN, sl]

                        # normalizers: ones-matmul column sums, replicated
                        for x, R in ((0, RA), (1, RB)):
                            for ch in range(NCH):
                                sl = slice(ch * CHUNK, (ch + 1) * CHUNK)
                                psn = ps_n.tile([128, CHUNK], F32, tag="psn")
                                for mt in range(TN):
                                    nc.tensor.matmul(psn, ones, eslice(x, mt, sl),
                                                     start=(mt == 0),
                                                     stop=(mt == TN - 1))
                                nc.vector.reciprocal(out=R[:, sl], in_=psn)
                        # A @ V, column-packed head pair; eviction folds R in
                        for ch in range(NCH):
                            sl = slice(ch * CHUNK, (ch + 1) * CHUNK)
                            psoA = ps_o.tile([128, CHUNK], F32, tag="pso",
                                             name="psoA")
                            psoB = ps_o.tile([128, CHUNK], F32, tag="pso",
                                             name="psoB")
                            for mt in range(TN):
                                nc.tensor.matmul(
                                    psoA[0:64, :], V[:, mt, hA * 64:hA * 64 + 64],
                                    eslice(0, mt, sl), start=(mt == 0),
                                    stop=(mt == TN - 1), tile_position=(0, 0))
                                nc.tensor.matmul(
                                    psoB[64:128, :], V[:, mt, hB * 64:hB * 64 + 64],
                                    eslice(1, mt, sl), start=(mt == 0),
                                    stop=(mt == TN - 1), tile_position=(0, 64))
                            nc.vector.tensor_tensor(
                                out=OT[0:64, p, sl], in0=psoA[0:64, :],
                                in1=RA[0:64, sl], op=ALU.mult)
                            nc.vector.tensor_tensor(
                                out=OT[64:128, p, sl], in0=psoB[64:128, :],
                                in1=RB[64:128, sl], op=ALU.mult)

                # --- output projection + residual accumulate into x_mid ---
                w_sb = wpool.tile([128, TC, C], BF16, tag="W")
                wg = wp[br].rearrange("(k p) c -> p k c", p=128)
                for k in range(TC):
                    nc.sync.dma_start(out=w_sb[:, k, :], in_=wg[:, k, :])
                if no_wp:
                    if br == 1:
                        nc.sync.dma_start(out=x_mid, in_=x1)
                    continue
                with tc.tile_pool(name=f"ps_b{br}", bufs=6, space="PSUM") as psp:
                    for nt in range(TN):
                        for ch in range(NWC):
                            sl = slice(ch * WCH, (ch + 1) * WCH)
                            ps = psp.tile([128, WCH], F32, tag="ps")
                            for j in range(TC):
                                nc.tensor.matmul(
                                    ps, OT[:, j, nt * 128:(nt + 1) * 128],
                                    w_sb[:, j, sl], start=(j == 0), stop=False)
                            brow = battn_sb if br == 0 else zrow
                            nc.tensor.matmul(ps, ones[0:1, :], brow[0:1, sl],
                                             start=False, stop=True)
                            xr = work.tile([128, WCH], F32, tag="resid")
                            src = x1 if br == 0 else x_mid
                            nc.sync.dma_start(
                                out=xr, in_=src[nt * 128:(nt + 1) * 128, sl])
                            st = work.tile([128, WCH], F32, tag="stage")
                            nc.vector.tensor_tensor(out=st, in0=ps, in1=xr,
                                                    op=ALU.add)
                            nc.sync.dma_start(
                                out=x_mid[nt * 128:(nt + 1) * 128, sl], in_=st)

        # ---- MLP ----
        if no_mlp:
            nc.sync.dma_start(out=out, in_=x_mid)
            return
        with ExitStack() as mctx:
            mlp = mctx.enter_context(tc.tile_pool(name="mlp", bufs=1))
            xnT = glob.tile([128, TC, B_N], BF16, tag="xnkvT")
            if no_ln:
                nc.vector.memset(xnT, 0.125)
            with tc.tile_pool(name="ps_ln3", bufs=4, space="PSUM") as ps_t:
                layer_norm_T(x_mid, xnT, ps_t)
            hT = mlp.tile([128, TH, B_N], BF16, tag="hT")
            wf_sb = mlp.tile([128, TC, HID], BF16, tag="Wfc")
            wg1 = wfc1.rearrange("(k p) c -> p k c", p=128)
            for k in range(TC):
                nc.sync.dma_start(out=wf_sb[:, k, :], in_=wg1[:, k, :])
            with tc.tile_pool(name="ps_m1", bufs=6, space="PSUM") as psp:
                for ht in range(TH):
                    for ch in range(NCH):
                        sl = slice(ch * CHUNK, (ch + 1) * CHUNK)
                        ps = psp.tile([128, CHUNK], F32, tag="ps")
                        for k in range(TC):
                            nc.tensor.matmul(
                                ps, wf_sb[:, k, ht * 128:(ht + 1) * 128],
                                xnT[:, k, sl], start=(k == 0), stop=(k == TC - 1))
                        nc.scalar.activation(out=hT[:, ht, sl], in_=ps,
                                             func=getattr(ACT, g["act"]),
                                             bias=hb_sb[:, ht:ht + 1], scale=1.0)
            wf2_sb = mlp.tile([128, TH, C], BF16, tag="Wfc")
            wg2 = wfc2.rearrange("(k p) c -> p k c", p=128)
            for k in range(TH):
                nc.sync.dma_start(out=wf2_sb[:, k, :], in_=wg2[:, k, :])
            with tc.tile_pool(name="ps_m2", bufs=6, space="PSUM") as psp:
                for nt in range(TN):
                    for ch in range(NWC):
                        sl = slice(ch * WCH, (ch + 1) * WCH)
                        ps = psp.tile([128, WCH], F32, tag="ps")
                        for k in range(TH):
                            nc.tensor.matmul(
                                ps, hT[:, k, nt * 128:(nt + 1) * 128],
                                wf2_sb[:, k, sl], start=(k == 0), stop=False)
                        nc.tensor.matmul(ps, ones[0:1, :], fbias_sb[0:1, sl],
                                         start=False, stop=True)
                        xr = work.tile([128, WCH], F32, tag="resid")
                        nc.sync.dma_start(
                            out=xr, in_=x_mid[nt * 128:(nt + 1) * 128, sl])
                        st = work.tile([128, WCH], F32, tag="stage")
                        nc.vector.tensor_tensor(out=st, in0=ps, in1=xr, op=ALU.add)
                        nc.sync.dma_start(
                            out=out[nt * 128:(nt + 1) * 128, sl], in_=st)


# ---------------- host side ----------------

_NC_CACHE = {}


def _get_nc(n_iters=1, **kw):
    key = (n_iters, tuple(sorted(kw.items())))
    if key not in _NC_CACHE:
        _NC_CACHE[key] = build_nc(n_iters=n_iters, **kw)
    return _NC_CACHE[key]


def preprocess(inputs, np_mod=np):
    """Fold LN affine params into weights / bias vectors; cast to bf16."""
    f = {k: np.asarray(v, dtype=np.float32) for k, v in inputs.items()}
    bf = ml_dtypes.bfloat16

    def fold(g, w):
        return (g[:, None] * w)

    shared = {}
    for i, (lg, lb, kvg, kvb, pre) in enumerate((
            ("ln11_g", "ln11_b", "ln12_g", "ln12_b", "a1"),
            ("ln21_g", "ln21_b", "ln23_g", "ln23_b", "a2"))):
        wqf = fold(f[lg], f[f"{pre}_wq"])
        wkf = fold(f[kvg], f[f"{pre}_wk"])
        wvf = fold(f[kvg], f[f"{pre}_wv"])
        shared[f"wq{i+1}"] = wqf.astype(bf)
        shared[f"wk{i+1}"] = wkf.astype(bf)
        shared[f"wv{i+1}"] = wvf.astype(bf)
        shared[f"wp{i+1}"] = f[f"{pre}_wp"].astype(bf)
        shared[f"qb{i+1}"] = (f[lb] @ f[f"{pre}_wq"]).astype(np.float32)
        shared[f"kb{i+1}"] = (f[kvb] @ f[f"{pre}_wk"]).astype(np.float32)
    vb1 = f["ln12_b"] @ f["a1_wv"]
    vb2 = f["ln23_b"] @ f["a2_wv"]
    shared["battn"] = (vb1 @ f["a1_wp"] + f["a1_bp"]
                       + vb2 @ f["a2_wp"] + f["a2_bp"]).astype(bf)
    shared["wfc1"] = fold(f["ln2_g"], f["fc1_w"]).astype(bf)
    shared["hb"] = (f["ln2_b"] @ f["fc1_w"] + f["fc1_b"]).astype(np.float32)
    shared["wfc2"] = f["fc2_w"].astype(bf)
    shared["fbias"] = f["fc2_b"].astype(bf)
    return shared


def kernel(**inputs):
    nc = _get_nc()
    shared = preprocess(inputs)
    x1 = np.asarray(inputs["x_1"], np.float32)
    x2 = np.asarray(inputs["x_2"], np.float32)
    x3 = np.asarray(inputs["x_3"], np.float32)
    B = x1.shape[0]
    assert B == N_CORES
    in_maps = [dict(shared, x1=x1[b], x2=x2[b], x3=x3[b]) for b in range(B)]
    res = run_bass_kernel_spmd(nc, in_maps, list(range(N_CORES)))
    return np.stack([res.results[b]["out"] for b in range(B)]).astype(np.float32)

